# revision 4
# baseline (speedup 1.0000x reference)
"""Multi-head attention block on 8 TRN2 NeuronCores.

Problem: x[2,2048,768] -> qkv proj -> 12-head attention -> out proj.
Sharding: 24 (batch, head) pairs across 8 cores; core c handles batch
c//4 and heads 3*(c%4)..3*(c%4)+2. Each core computes its heads'
Q,K,V, attention, and a partial output projection; the host sums the
four per-batch partials and adds the bias terms.

Design notes (v7):
  - Row-tiled QK: the QK^T contraction is only 64 (head dim), so two
    64-row PE tiles run concurrently (tile_position inferred from
    base_partition 0/64). Heads h0+h1 pair up per query block; h2
    pairs its own two query blocks. ~2x QK matmul throughput.
  - Exp split across engines: ACT does stream-A exp natively; DVE
    does stream-B via a Schraudolph bit trick (one tensor_scalar:
    int16(s*184.665 + 16249) bitcast to bf16, ~3% elementwise error
    that mostly cancels in softmax).
  - Sweep order h2-first (both blocks), then h0h1 block0, then the
    block0 output projection, then h0h1 block1 - so most of the
    projection overlaps attention and the serial tail shrinks.
  - Fused weight columns [K2|Q2 | K0|K1 | Q0|Q1] make every
    PSUM->SBUF write a single aligned copy (K0 top / K1 bottom is
    exactly the packed layout the row-tiled QK wants).
  - All matmul operands bf16; output bf16; host sums partials in f32.
"""

import os
import sys

for _p in ("/opt/trn_rl_repo", "/opt/pypackages"):
    if _p not in sys.path:
        sys.path.append(_p)

import numpy as np

B, N, C = 2, 2048, 768
H, D = 12, 64
HPC = 3                    # heads per core
J = HPC * D                # 192 per-core head-dim rows
NCORES = 8
MC = N // 128              # 16 key chunks
KC = C // 128              # 6 contraction chunks for projections
NWARM = 6

SCH_A = 184.6649652337873   # 2^7 / ln 2
SCH_B = 16249.0             # exponent-bias offset, tuned for softmax

_cache = {}
LAST_RESULTS = None


def _build():
    import concourse.mybir as mybir
    import concourse.tile as tile
    from concourse import bacc

    f32 = mybir.dt.float32
    bf16 = mybir.dt.bfloat16
    i16 = mybir.dt.int16
    Exp = mybir.ActivationFunctionType.Exp
    Copy = mybir.ActivationFunctionType.Copy
    mult = mybir.AluOpType.mult
    add = mybir.AluOpType.add

    nc = bacc.Bacc("TRN2", target_bir_lowering=False, debug=False,
                   num_devices=NCORES)

    xt_d = nc.declare_dram_parameter("xt", [C, N], bf16, isOutput=False)
    # fused weight columns [K2|Q2 | K0|K1 | Q0|Q1]
    wkq_d = nc.declare_dram_parameter("wkq", [C, 2 * J], bf16,
                                      isOutput=False)
    wv_d = nc.declare_dram_parameter("wv", [C, J], bf16, isOutput=False)
    bqA_d = nc.declare_dram_parameter("bqA", [128, 1], f32, isOutput=False)
    bq2_d = nc.declare_dram_parameter("bq2", [64, 1], f32, isOutput=False)
    # padded proj weights: rows 0:128 = heads 0,1; 128:192 = head 2;
    # 192:256 = zero (annihilates ah2[1]'s junk bottom half)
    wp_d = nc.declare_dram_parameter("wp", [2 * 128, C], bf16,
                                     isOutput=False)
    out_d = nc.declare_dram_parameter("out", [N, C], bf16, isOutput=True)

    with tile.TileContext(nc) as tc:
        with (
            tc.tile_pool(name="persist", bufs=1) as pp,
            tc.tile_pool(name="osb", bufs=4) as posb,
            tc.tile_pool(name="etile", bufs=4) as pe,
            tc.tile_pool(name="bcsb", bufs=2) as pbc,
        ):
            warm_t = pp.tile([128, 512], bf16, tag="warm_t", name="warm_t")
            wkq = [pp.tile([128, 2 * J], bf16, tag=f"wkq{k}",
                           name=f"wkq{k}") for k in range(KC)]
            xt = [pp.tile([128, N], bf16, tag=f"xt{k}", name=f"xt{k}")
                  for k in range(KC)]
            wv = [pp.tile([128, J], bf16, tag=f"wv{k}", name=f"wv{k}")
                  for k in range(KC)]
            bqA = pp.tile([128, 1], f32, tag="bqA", name="bqA")
            bq2 = pp.tile([64, 1], f32, tag="bq2", name="bq2")
            wp = [pp.tile([128, C], bf16, tag=f"wp{t}", name=f"wp{t}")
                  for t in range(2)]
            # K^T packed for row tiling: khA = (K0 top, K1 bottom),
            # kh2 = (K2 top, K2 duplicated bottom)
            khA = pp.tile([128, N], bf16, tag="khA", name="khA")
            kh2 = pp.tile([128, N], bf16, tag="kh2", name="kh2")
            # Q^T packed likewise: qA = (Q0 top, Q1 bottom); q2 carries
            # Q2 in the top half for query block0 cols and in the
            # bottom half for block1 cols (other quadrants never read)
            qA = pp.tile([128, N], bf16, tag="qA", name="qA")
            q2 = pp.tile([128, N], bf16, tag="q2", name="q2")
            # V with a ones column per head: [128, 3*65]
            vx = [pp.tile([128, HPC * 65], bf16, tag=f"vx{m}",
                          name=f"vx{m}") for m in range(MC)]
            sums = [pp.tile([1, N], f32, tag=f"sums{h}", name=f"sums{h}")
                    for h in range(HPC)]
            raw = [pp.tile([64, N], bf16, tag=f"raw{h}", name=f"raw{h}")
                   for h in range(HPC)]
            ah2 = [pp.tile([128, N], bf16, tag=f"ah2{t}", name=f"ah2{t}")
                   for t in range(2)]

            # ---- constants via DVE memset (no DMA traffic) ----
            nc.vector.memset(warm_t[:], 1.0)
            nc.vector.memset(ah2[1][64:128, :], 0.0)
            for m in range(MC):
                on = vx[m].rearrange("p (h e) -> p h e", e=65)[:, :, 64:65]
                nc.vector.memset(on, 1.0)

            # ---- input DMA: x^T halves round-robin on two queues,
            # weights on the scalar queue ----
            dq = [nc.sync, nc.gpsimd]
            for half in range(2):
                csl = slice(1024 * half, 1024 * (half + 1))
                for k in range(KC):
                    dq[k % 2].dma_start(xt[k][:, csl],
                                        xt_d[128 * k:128 * (k + 1), csl])
            for t in range(2):
                nc.sync.dma_start(wp[t][:], wp_d[128 * t:128 * (t + 1), :])
            nc.scalar.dma_start(bqA[:], bqA_d[:, :])
            nc.scalar.dma_start(bq2[:], bq2_d[:, :])
            for k in range(KC):
                nc.scalar.dma_start(wkq[k][:], wkq_d[128 * k:128 * (k + 1), :])
            for k in range(KC):
                nc.scalar.dma_start(wv[k][:], wv_d[128 * k:128 * (k + 1), :])

            with tc.tile_pool(name="ps1", bufs=1, space="PSUM") as ps1:
                # PE p-state warmup during the DMA prologue
                for i in range(NWARM):
                    ps = ps1.tile([128, 512], f32, tag="qk", bufs=2,
                                  name=f"warm{i}")
                    nc.tensor.matmul(ps[:], warm_t[:, 0:128], warm_t[:])

                def g_group(b, g):
                    # g0 -> [K2;Q2], g1 -> [K0;K1], g2 -> [Q0;Q1]
                    nsl = slice(512 * b, 512 * (b + 1))
                    ps = ps1.tile([128, 512], f32, tag="qk", bufs=2,
                                  name="ps_qk")
                    for k in range(KC):
                        nc.tensor.matmul(
                            ps[:], wkq[k][:, 128 * g:128 * (g + 1)],
                            xt[k][:, nsl],
                            start=(k == 0), stop=(k == KC - 1))
                    if g == 0:
                        nc.vector.tensor_copy(kh2[0:64, nsl], ps[0:64, :])
                        nc.vector.tensor_copy(kh2[64:128, nsl], ps[0:64, :])
                        qr = slice(0, 64) if b < 2 else slice(64, 128)
                        nc.vector.tensor_scalar(
                            q2[qr, nsl], ps[64:128, :], 0.125,
                            bq2[:], mult, add)
                    elif g == 1:
                        nc.vector.tensor_copy(khA[:, nsl], ps[:])
                    else:
                        nc.vector.tensor_scalar(
                            qA[:, nsl], ps[:], 0.125, bqA[:], mult, add)

                def v_chunk(m):
                    msl = slice(128 * m, 128 * (m + 1))
                    ps = ps1.tile([128, 512], f32, tag="qk", bufs=2,
                                  name="ps_v")
                    for k in range(KC):
                        nc.tensor.matmul(ps[:, 0:J], xt[k][:, msl], wv[k][:],
                                         start=(k == 0), stop=(k == KC - 1))
                    vdst = vx[m].rearrange("p (h e) -> p h e",
                                           e=65)[:, :, 0:64]
                    nc.vector.tensor_copy(
                        vdst, ps[:, 0:J].rearrange("p (h e) -> p h e", e=64))

                # first-half work while second halves stream in
                for b in (0, 1):
                    for g in range(3):
                        g_group(b, g)
                for m in range(8):
                    v_chunk(m)
                for b in (2, 3):
                    for g in range(3):
                        g_group(b, g)
                for m in range(8, MC):
                    v_chunk(m)

            with tc.tile_pool(name="ps2", bufs=1, space="PSUM") as ps2:

                def flush(h, cols, av, on_act, adst, r0):
                    # raw copy first (it alone gates the next sweep's
                    # AV PSUM writes), then sums + normalize chain off
                    # the critical path
                    if on_act:
                        nc.scalar.activation(raw[h][:, cols],
                                             av[0:64, :], Copy)
                    else:
                        nc.vector.tensor_copy(raw[h][:, cols],
                                              av[0:64, :])
                    nc.vector.tensor_copy(sums[h][:, cols], av[64:65, :])
                    for i in range(2):
                        hf = slice(cols.start + 512 * i,
                                   cols.start + 512 * (i + 1))
                        bcs = pbc.tile([64, 512], f32, tag="bcs",
                                       name="bcs")
                        nc.gpsimd.partition_broadcast(bcs[:],
                                                      sums[h][:, hf])
                        rec = pbc.tile([64, 512], f32, tag="rec",
                                       name="rec")
                        nc.vector.reciprocal_approx_fast(rec[:], bcs[:])
                        nc.vector.tensor_mul(adst[r0:r0 + 64, hf],
                                             raw[h][:, hf], rec[:])

                def sweep(ktile, qtile, colsA, colsB, vslA, vslB):
                    avA = ps2.tile([65, 1024], f32, tag="avA", bufs=1,
                                   name="ps_avA")
                    avB = ps2.tile([65, 1024], f32, tag="avB", bufs=1,
                                   name="ps_avB")
                    for m in range(MC):
                        msl = slice(128 * m, 128 * (m + 1))
                        sA = ps2.tile([128, 1024], f32, tag="sA", bufs=1,
                                      name="ps_sA")
                        sB = ps2.tile([128, 1024], f32, tag="sB", bufs=1,
                                      name="ps_sB")
                        # interleave the two row tiles so consecutive
                        # matmuls hit disjoint row groups and overlap
                        for i in range(2):
                            nc.tensor.matmul(
                                sA[:, 512 * i:512 * (i + 1)],
                                ktile[0:64, msl],
                                qtile[0:64, colsA.start + 512 * i:
                                      colsA.start + 512 * (i + 1)])
                            nc.tensor.matmul(
                                sB[:, 512 * i:512 * (i + 1)],
                                ktile[64:128, msl],
                                qtile[64:128, colsB.start + 512 * i:
                                      colsB.start + 512 * (i + 1)])
                        e0 = pe.tile([128, 1024], bf16, tag="e0", name="e0")
                        nc.scalar.activation(e0[:], sA[:], Exp)
                        e1 = pe.tile([128, 1024], bf16, tag="e1", name="e1")
                        nc.vector.tensor_scalar(e1.bitcast(i16)[:], sB[:],
                                                SCH_A, SCH_B, mult, add)
                        for i in range(2):
                            nc.tensor.matmul(
                                avA[:, 512 * i:512 * (i + 1)],
                                vx[m][:, vslA],
                                e0[:, 512 * i:512 * (i + 1)],
                                start=(m == 0), stop=(m == MC - 1))
                        for i in range(2):
                            nc.tensor.matmul(
                                avB[:, 512 * i:512 * (i + 1)],
                                vx[m][:, vslB],
                                e1[:, 512 * i:512 * (i + 1)],
                                start=(m == 0), stop=(m == MC - 1))
                    return avA, avB

                def proj(ms):
                    for m in ms:
                        msl = slice(128 * m, 128 * (m + 1))
                        t = ps2.tile([128, 1024], f32,
                                     tag=("sA" if m % 2 == 0 else "sB"),
                                     bufs=1, name="ps_pj")
                        for tt in range(2):
                            nc.tensor.matmul(t[:, 0:512], ah2[tt][:, msl],
                                             wp[tt][:, 0:512],
                                             start=(tt == 0), stop=(tt == 1))
                        for tt in range(2):
                            nc.tensor.matmul(t[:, 512:768], ah2[tt][:, msl],
                                             wp[tt][:, 512:768],
                                             start=(tt == 0), stop=(tt == 1))
                        o3 = posb.tile([128, C], bf16, tag="o3", name="o3")
                        if m % 2 == 0:
                            nc.vector.tensor_copy(o3[:], t[:, 0:768])
                        else:
                            nc.scalar.activation(o3[:], t[:, 0:768], Copy)
                        nc.sync.dma_start(out_d[msl, :], o3[:])

                v2 = slice(130, 195)
                v0 = slice(0, 65)
                v1 = slice(65, 130)
                c0 = slice(0, 1024)
                c1 = slice(1024, 2048)

                # sweep 1: head 2, both query blocks via row tiling
                avA, avB = sweep(kh2, q2, c0, c1, v2, v2)
                flush(2, c0, avA, True, ah2[1], 0)
                flush(2, c1, avB, False, ah2[1], 0)
                # sweep 2: heads 0+1, query block 0
                avA, avB = sweep(khA, qA, c0, c0, v0, v1)
                flush(0, c0, avA, True, ah2[0], 0)
                flush(1, c0, avB, False, ah2[0], 64)
                # block-0 projection overlaps the tail flushes
                proj(range(8))
                # sweep 3: heads 0+1, query block 1
                avA, avB = sweep(khA, qA, c1, c1, v0, v1)
                flush(0, c1, avA, True, ah2[0], 0)
                flush(1, c1, avB, False, ah2[0], 64)
                proj(range(8, MC))

    nc.compile()
    return nc


def kernel(x, w_qkv, b_qkv, w_proj, b_proj):
    import ml_dtypes

    from concourse.bass_utils import run_bass_kernel_spmd

    global LAST_RESULTS
    if "nc" not in _cache:
        _cache["nc"] = _build()
    nc = _cache["nc"]

    bf16 = ml_dtypes.bfloat16
    x = np.asarray(x, dtype=np.float32)
    w_qkv = np.asarray(w_qkv, dtype=np.float32)
    b_qkv = np.asarray(b_qkv, dtype=np.float32)
    w_proj = np.asarray(w_proj, dtype=np.float32)
    b_proj = np.asarray(b_proj, dtype=np.float32)

    in_maps = []
    for c in range(NCORES):
        b = c // 4
        h0 = HPC * (c % 4)
        q = [w_qkv[:, 64 * (h0 + h):64 * (h0 + h + 1)] for h in range(HPC)]
        k = [w_qkv[:, C + 64 * (h0 + h):C + 64 * (h0 + h + 1)]
             for h in range(HPC)]
        vs = slice(2 * C + 64 * h0, 2 * C + 64 * (h0 + HPC))
        wkq = np.concatenate([k[2], q[2], k[0], k[1], q[0], q[1]], axis=1)
        bq = b_qkv[64 * h0:64 * (h0 + HPC)] * 0.125
        wp_pad = np.zeros((2 * 128, C), dtype=np.float32)
        wp_pad[0:128] = w_proj[64 * h0:64 * (h0 + 2), :]
        wp_pad[128:192] = w_proj[64 * (h0 + 2):64 * (h0 + 3), :]
        in_maps.append({
            "xt": np.ascontiguousarray(x[b].T).astype(bf16),
            "wkq": np.ascontiguousarray(wkq).astype(bf16),
            "wv": np.ascontiguousarray(w_qkv[:, vs]).astype(bf16),
            "bqA": np.ascontiguousarray(bq[0:128].reshape(128, 1)).astype(
                np.float32),
            "bq2": np.ascontiguousarray(bq[128:192].reshape(64, 1)).astype(
                np.float32),
            "wp": wp_pad.astype(bf16),
        })

    res = run_bass_kernel_spmd(nc, in_maps, core_ids=list(range(NCORES)))
    LAST_RESULTS = res

    out = np.zeros((B, N, C), dtype=np.float32)
    for c in range(NCORES):
        out[c // 4] += np.asarray(res.results[c]["out"], dtype=np.float32)
    out += b_proj + b_qkv[2 * C:] @ w_proj
    return out


# revision 9
# speedup vs baseline: 1.3232x; 1.3232x over previous
"""Multi-head attention block on 8 TRN2 NeuronCores.

Problem: x[2,2048,768] -> qkv proj -> 12-head attention -> out proj.
Sharding: 24 (batch, head) pairs across 8 cores; core c handles batch
c//4 and heads 3*(c%4)..3*(c%4)+2. Each core computes its heads'
Q,K,V, attention, and a partial output projection; the host sums the
four per-batch partials and adds the bias terms.

Design notes (v7):
  - Row-tiled QK: the QK^T contraction is only 64 (head dim), so two
    64-row PE tiles run concurrently (tile_position inferred from
    base_partition 0/64). Heads h0+h1 pair up per query block; h2
    pairs its own two query blocks. ~2x QK matmul throughput.
  - Exp split across engines: ACT does stream-A exp natively; DVE
    does stream-B via a Schraudolph bit trick (one tensor_scalar:
    int16(s*184.665 + 16249) bitcast to bf16, ~3% elementwise error
    that mostly cancels in softmax).
  - Sweep order h2-first (both blocks), then h0h1 block0, then the
    block0 output projection, then h0h1 block1 - so most of the
    projection overlaps attention and the serial tail shrinks.
  - Fused weight columns [K2|Q2 | K0|K1 | Q0|Q1] make every
    PSUM->SBUF write a single aligned copy (K0 top / K1 bottom is
    exactly the packed layout the row-tiled QK wants).
  - All matmul operands bf16; output bf16; host sums partials in f32.
"""

import os
import sys

for _p in ("/opt/trn_rl_repo", "/opt/pypackages"):
    if _p not in sys.path:
        sys.path.append(_p)

import numpy as np

B, N, C = 2, 2048, 768
H, D = 12, 64
HPC = 3                    # heads per core
J = HPC * D                # 192 per-core head-dim rows
NCORES = 8
MC = N // 128              # 16 key chunks
KC = C // 128              # 6 contraction chunks for projections
NWARM = 6

SCH_A = 184.6649652337873   # 2^7 / ln 2
SCH_B = 16249.0             # exponent-bias offset, tuned for softmax

_cache = {}
LAST_RESULTS = None


def _build():
    import concourse.mybir as mybir
    import concourse.tile as tile
    from concourse import bacc

    f32 = mybir.dt.float32
    bf16 = mybir.dt.bfloat16
    i16 = mybir.dt.int16
    Exp = mybir.ActivationFunctionType.Exp
    Copy = mybir.ActivationFunctionType.Copy
    mult = mybir.AluOpType.mult
    add = mybir.AluOpType.add

    nc = bacc.Bacc("TRN2", target_bir_lowering=False, debug=False,
                   num_devices=NCORES)

    xt_d = nc.declare_dram_parameter("xt", [C, N], bf16, isOutput=False)
    # fused weight columns [K2|Q2 | K0|K1 | Q0|Q1]
    wkq_d = nc.declare_dram_parameter("wkq", [C, 2 * J], bf16,
                                      isOutput=False)
    wv_d = nc.declare_dram_parameter("wv", [C, J], bf16, isOutput=False)
    bqA_d = nc.declare_dram_parameter("bqA", [128, 1], f32, isOutput=False)
    bq2_d = nc.declare_dram_parameter("bq2", [64, 1], f32, isOutput=False)
    # padded proj weights: rows 0:128 = heads 0,1; 128:192 = head 2;
    # 192:256 = zero (annihilates ah2[1]'s junk bottom half)
    wp_d = nc.declare_dram_parameter("wp", [2 * 128, C], bf16,
                                     isOutput=False)
    out_d = nc.declare_dram_parameter("out", [N, C], bf16, isOutput=True)

    with tile.TileContext(nc) as tc:
        with (
            tc.tile_pool(name="persist", bufs=1) as pp,
            tc.tile_pool(name="osb", bufs=4) as posb,
            tc.tile_pool(name="etile", bufs=4) as pe,
            tc.tile_pool(name="bcsb", bufs=2) as pbc,
        ):
            warm_t = pp.tile([128, 512], bf16, tag="warm_t", name="warm_t")
            wkq = [pp.tile([128, 2 * J], bf16, tag=f"wkq{k}",
                           name=f"wkq{k}") for k in range(KC)]
            xt = [pp.tile([128, N], bf16, tag=f"xt{k}", name=f"xt{k}")
                  for k in range(KC)]
            wv = [pp.tile([128, J], bf16, tag=f"wv{k}", name=f"wv{k}")
                  for k in range(KC)]
            bqA = pp.tile([128, 1], f32, tag="bqA", name="bqA")
            bq2 = pp.tile([64, 1], f32, tag="bq2", name="bq2")
            wp = [pp.tile([128, C], bf16, tag=f"wp{t}", name=f"wp{t}")
                  for t in range(2)]
            # K^T per head, zero rows alternating so the packed Q tiles
            # need none: kh0 data 0:64, kh1 data 64:128, kh2 data 0:64
            kh = [pp.tile([128, N], bf16, tag=f"kh{h}", name=f"kh{h}")
                  for h in range(HPC)]
            # Q^T packed: qA = (Q0 top, Q1 bottom); qB = (Q2 top, junk)
            qA = pp.tile([128, N], bf16, tag="qA", name="qA")
            qB = pp.tile([128, N], bf16, tag="qB", name="qB")
            # V with a ones column per head: [128, 3*65]
            vx = [pp.tile([128, HPC * 65], bf16, tag=f"vx{m}",
                          name=f"vx{m}") for m in range(MC)]
            sums = [pp.tile([1, N], f32, tag=f"sums{h}", name=f"sums{h}")
                    for h in range(HPC)]
            raw = [pp.tile([64, N], bf16, tag=f"raw{h}", name=f"raw{h}")
                   for h in range(HPC)]
            ah2 = [pp.tile([128, N], bf16, tag=f"ah2{t}", name=f"ah2{t}")
                   for t in range(2)]

            # ---- constants via DVE memset (no DMA traffic) ----
            nc.vector.memset(warm_t[:], 1.0)
            nc.vector.memset(kh[0][64:128, :], 0.0)
            nc.vector.memset(kh[1][0:64, :], 0.0)
            nc.vector.memset(kh[2][64:128, :], 0.0)
            nc.vector.memset(qB[64:128, :], 0.0)
            nc.vector.memset(ah2[1][64:128, :], 0.0)
            for m in range(MC):
                on = vx[m].rearrange("p (h e) -> p h e", e=65)[:, :, 64:65]
                nc.vector.memset(on, 1.0)

            # ---- input DMA: x^T halves round-robin on two queues,
            # weights on the scalar queue ----
            dq = [nc.sync, nc.gpsimd]
            for half in range(2):
                csl = slice(1024 * half, 1024 * (half + 1))
                for k in range(KC):
                    dq[k % 2].dma_start(xt[k][:, csl],
                                        xt_d[128 * k:128 * (k + 1), csl])
            for t in range(2):
                nc.sync.dma_start(wp[t][:], wp_d[128 * t:128 * (t + 1), :])
            nc.scalar.dma_start(bqA[:], bqA_d[:, :])
            nc.scalar.dma_start(bq2[:], bq2_d[:, :])
            for k in range(KC):
                nc.scalar.dma_start(wkq[k][:], wkq_d[128 * k:128 * (k + 1), :])
            for k in range(KC):
                nc.scalar.dma_start(wv[k][:], wv_d[128 * k:128 * (k + 1), :])

            with tc.tile_pool(name="ps1", bufs=1, space="PSUM") as ps1:
                # PE p-state warmup during the DMA prologue
                for i in range(NWARM):
                    ps = ps1.tile([128, 512], f32, tag="qk", bufs=2,
                                  name=f"warm{i}")
                    nc.tensor.matmul(ps[:], warm_t[:, 0:128], warm_t[:])

                def g_group(b, g):
                    # g0 -> [K2;Q2], g1 -> [K0;K1], g2 -> [Q0;Q1]
                    nsl = slice(512 * b, 512 * (b + 1))
                    ps = ps1.tile([128, 512], f32, tag="qk", bufs=2,
                                  name="ps_qk")
                    for k in range(KC):
                        nc.tensor.matmul(
                            ps[:], wkq[k][:, 128 * g:128 * (g + 1)],
                            xt[k][:, nsl],
                            start=(k == 0), stop=(k == KC - 1))
                    if g == 0:
                        nc.vector.tensor_copy(kh[2][0:64, nsl], ps[0:64, :])
                        nc.vector.tensor_scalar(
                            qB[0:64, nsl], ps[64:128, :], 0.125,
                            bq2[:], mult, add)
                    elif g == 1:
                        nc.vector.tensor_copy(kh[0][0:64, nsl], ps[0:64, :])
                        nc.vector.tensor_copy(kh[1][64:128, nsl],
                                              ps[64:128, :])
                    else:
                        nc.vector.tensor_scalar(
                            qA[:, nsl], ps[:], 0.125, bqA[:], mult, add)

                def v_chunk(m):
                    msl = slice(128 * m, 128 * (m + 1))
                    ps = ps1.tile([128, 512], f32, tag="qk", bufs=2,
                                  name="ps_v")
                    for k in range(KC):
                        nc.tensor.matmul(ps[:, 0:J], xt[k][:, msl], wv[k][:],
                                         start=(k == 0), stop=(k == KC - 1))
                    vdst = vx[m].rearrange("p (h e) -> p h e",
                                           e=65)[:, :, 0:64]
                    nc.vector.tensor_copy(
                        vdst, ps[:, 0:J].rearrange("p (h e) -> p h e", e=64))

                # first-half work while second halves stream in
                for b in (0, 1):
                    for g in range(3):
                        g_group(b, g)
                for m in range(8):
                    v_chunk(m)
                for b in (2, 3):
                    for g in range(3):
                        g_group(b, g)
                for m in range(8, MC):
                    v_chunk(m)

            with tc.tile_pool(name="ps2", bufs=1, space="PSUM") as ps2:

                def flush(h, cols, av, on_act, adst, r0):
                    # raw copy first (it alone gates the next sweep's
                    # AV PSUM writes), then sums + normalize chain off
                    # the critical path
                    if on_act:
                        nc.scalar.activation(raw[h][:, cols],
                                             av[0:64, :], Copy)
                    else:
                        nc.vector.tensor_copy(raw[h][:, cols],
                                              av[0:64, :])
                    nc.vector.tensor_copy(sums[h][:, cols], av[64:65, :])
                    for i in range(2):
                        hf = slice(cols.start + 512 * i,
                                   cols.start + 512 * (i + 1))
                        bcs = pbc.tile([64, 512], f32, tag="bcs",
                                       name="bcs")
                        nc.gpsimd.partition_broadcast(bcs[:],
                                                      sums[h][:, hf])
                        rec = pbc.tile([64, 512], f32, tag="rec",
                                       name="rec")
                        nc.vector.reciprocal_approx_fast(rec[:], bcs[:])
                        nc.vector.tensor_mul(adst[r0:r0 + 64, hf],
                                             raw[h][:, hf], rec[:])

                def sweep(kA, qtA, colsA, kB, qtB, colsB, vslA, vslB):
                    avA = ps2.tile([65, 1024], f32, tag="avA", bufs=1,
                                   name="ps_avA")
                    avB = ps2.tile([65, 1024], f32, tag="avB", bufs=1,
                                   name="ps_avB")
                    for m in range(MC):
                        msl = slice(128 * m, 128 * (m + 1))
                        sA = ps2.tile([128, 1024], f32, tag="sA", bufs=1,
                                      name="ps_sA")
                        sB = ps2.tile([128, 1024], f32, tag="sB", bufs=1,
                                      name="ps_sB")
                        for i in range(2):
                            nc.tensor.matmul(
                                sA[:, 512 * i:512 * (i + 1)],
                                kA[:, msl],
                                qtA[:, colsA.start + 512 * i:
                                    colsA.start + 512 * (i + 1)])
                        for i in range(2):
                            nc.tensor.matmul(
                                sB[:, 512 * i:512 * (i + 1)],
                                kB[:, msl],
                                qtB[:, colsB.start + 512 * i:
                                    colsB.start + 512 * (i + 1)])
                        e0 = pe.tile([128, 1024], bf16, tag="e0", name="e0")
                        nc.scalar.activation(e0[:], sA[:], Exp)
                        e1 = pe.tile([128, 1024], bf16, tag="e1", name="e1")
                        nc.vector.tensor_scalar(e1.bitcast(i16)[:], sB[:],
                                                SCH_A, SCH_B, mult, add)
                        for i in range(2):
                            nc.tensor.matmul(
                                avA[:, 512 * i:512 * (i + 1)],
                                vx[m][:, vslA],
                                e0[:, 512 * i:512 * (i + 1)],
                                start=(m == 0), stop=(m == MC - 1))
                        for i in range(2):
                            nc.tensor.matmul(
                                avB[:, 512 * i:512 * (i + 1)],
                                vx[m][:, vslB],
                                e1[:, 512 * i:512 * (i + 1)],
                                start=(m == 0), stop=(m == MC - 1))
                    return avA, avB

                def proj(ms):
                    for m in ms:
                        msl = slice(128 * m, 128 * (m + 1))
                        t = ps2.tile([128, 1024], f32,
                                     tag=("sA" if m % 2 == 0 else "sB"),
                                     bufs=1, name="ps_pj")
                        for tt in range(2):
                            nc.tensor.matmul(t[:, 0:512], ah2[tt][:, msl],
                                             wp[tt][:, 0:512],
                                             start=(tt == 0), stop=(tt == 1))
                        for tt in range(2):
                            nc.tensor.matmul(t[:, 512:768], ah2[tt][:, msl],
                                             wp[tt][:, 512:768],
                                             start=(tt == 0), stop=(tt == 1))
                        o3 = posb.tile([128, C], bf16, tag="o3", name="o3")
                        if m % 2 == 0:
                            nc.vector.tensor_copy(o3[:], t[:, 0:768])
                        else:
                            nc.scalar.activation(o3[:], t[:, 0:768], Copy)
                        nc.sync.dma_start(out_d[msl, :], o3[:])

                v2 = slice(130, 195)
                v0 = slice(0, 65)
                v1 = slice(65, 130)
                c0 = slice(0, 1024)
                c1 = slice(1024, 2048)

                # sweep 1: head 2, both query blocks
                avA, avB = sweep(kh[2], qB, c0, kh[2], qB, c1, v2, v2)
                flush(2, c0, avA, True, ah2[1], 0)
                flush(2, c1, avB, False, ah2[1], 0)
                # sweep 2: heads 0+1, query block 0
                avA, avB = sweep(kh[0], qA, c0, kh[1], qA, c0, v0, v1)
                flush(0, c0, avA, True, ah2[0], 0)
                flush(1, c0, avB, False, ah2[0], 64)
                # block-0 projection overlaps the tail flushes
                proj(range(8))
                # sweep 3: heads 0+1, query block 1
                avA, avB = sweep(kh[0], qA, c1, kh[1], qA, c1, v0, v1)
                flush(0, c1, avA, True, ah2[0], 0)
                flush(1, c1, avB, False, ah2[0], 64)
                proj(range(8, MC))

    nc.compile()
    return nc


def kernel(x, w_qkv, b_qkv, w_proj, b_proj):
    import ml_dtypes

    from concourse.bass_utils import run_bass_kernel_spmd

    global LAST_RESULTS
    if "nc" not in _cache:
        _cache["nc"] = _build()
    nc = _cache["nc"]

    bf16 = ml_dtypes.bfloat16
    x = np.asarray(x, dtype=np.float32)
    w_qkv = np.asarray(w_qkv, dtype=np.float32)
    b_qkv = np.asarray(b_qkv, dtype=np.float32)
    w_proj = np.asarray(w_proj, dtype=np.float32)
    b_proj = np.asarray(b_proj, dtype=np.float32)

    in_maps = []
    for c in range(NCORES):
        b = c // 4
        h0 = HPC * (c % 4)
        q = [w_qkv[:, 64 * (h0 + h):64 * (h0 + h + 1)] for h in range(HPC)]
        k = [w_qkv[:, C + 64 * (h0 + h):C + 64 * (h0 + h + 1)]
             for h in range(HPC)]
        vs = slice(2 * C + 64 * h0, 2 * C + 64 * (h0 + HPC))
        wkq = np.concatenate([k[2], q[2], k[0], k[1], q[0], q[1]], axis=1)
        bq = b_qkv[64 * h0:64 * (h0 + HPC)] * 0.125
        wp_pad = np.zeros((2 * 128, C), dtype=np.float32)
        wp_pad[0:128] = w_proj[64 * h0:64 * (h0 + 2), :]
        wp_pad[128:192] = w_proj[64 * (h0 + 2):64 * (h0 + 3), :]
        in_maps.append({
            "xt": np.ascontiguousarray(x[b].T).astype(bf16),
            "wkq": np.ascontiguousarray(wkq).astype(bf16),
            "wv": np.ascontiguousarray(w_qkv[:, vs]).astype(bf16),
            "bqA": np.ascontiguousarray(bq[0:128].reshape(128, 1)).astype(
                np.float32),
            "bq2": np.ascontiguousarray(bq[128:192].reshape(64, 1)).astype(
                np.float32),
            "wp": wp_pad.astype(bf16),
        })

    res = run_bass_kernel_spmd(nc, in_maps, core_ids=list(range(NCORES)))
    LAST_RESULTS = res

    out = np.zeros((B, N, C), dtype=np.float32)
    for c in range(NCORES):
        out[c // 4] += np.asarray(res.results[c]["out"], dtype=np.float32)
    out += b_proj + b_qkv[2 * C:] @ w_proj
    return out


# revision 13
# speedup vs baseline: 1.4341x; 1.0838x over previous
"""Multi-head attention block on 8 TRN2 NeuronCores.

Problem: x[2,2048,768] -> qkv proj -> 12-head attention -> out proj.
Sharding: 24 (batch, head) pairs across 8 cores; core c handles batch
c//4 and heads 3*(c%4)..3*(c%4)+2. Each core computes its heads'
Q,K,V, attention, and a partial output projection; the host sums the
four per-batch partials and adds the bias terms.

Design notes (v7c):
  - Exp split across engines per key chunk: even chunks use the ACT
    engine's native Exp, odd chunks use a DVE Schraudolph bit trick
    (one tensor_scalar: int16(s*184.665 + 16249) bitcast to bf16).
    This removes the ACT engine as the attention bottleneck; the
    pipeline is PE-bound.
  - Single 1024-column moving matmuls for QK and AV (half the
    instruction count and LDWEIGHTS of the 512-col version).
  - One PSUM pool for all phases: s (2x[128,1024]) + v (2x[128,512])
    + av (1x[65,1024]) = 8 banks. This lets the V projection chunks
    4..15 interleave into the first attention pair and the block-0
    output projection interleave into pair (2,1).
  - Pair order (2,0),(0,0),(1,0),(2,1)+proj0,(0,1),(1,1),proj1 so
    only half the output projection remains in the serial tail.
  - Fused weight columns [K2|Q2 | K0|K1 | Q0|Q1]; K tiles carry the
    zero rows that annihilate the packed Q tiles' other-head rows.
  - All matmul operands bf16; output bf16; host sums partials in f32.
"""

import os
import sys

for _p in ("/opt/trn_rl_repo", "/opt/pypackages"):
    if _p not in sys.path:
        sys.path.append(_p)

import numpy as np

B, N, C = 2, 2048, 768
H, D = 12, 64
HPC = 3                    # heads per core
J = HPC * D                # 192 per-core head-dim rows
NCORES = 8
MC = N // 128              # 16 key chunks
KC = C // 128              # 6 contraction chunks for projections
NWARM = 6

SCH_A = 184.6649652337873   # 2^7 / ln 2
SCH_B = 16249.0             # exponent-bias offset, tuned for softmax

_cache = {}
LAST_RESULTS = None


def _build():
    import concourse.mybir as mybir
    import concourse.tile as tile
    from concourse import bacc

    f32 = mybir.dt.float32
    bf16 = mybir.dt.bfloat16
    i16 = mybir.dt.int16
    Exp = mybir.ActivationFunctionType.Exp
    Copy = mybir.ActivationFunctionType.Copy
    mult = mybir.AluOpType.mult
    add = mybir.AluOpType.add

    nc = bacc.Bacc("TRN2", target_bir_lowering=False, debug=False,
                   num_devices=NCORES)

    xt_d = nc.declare_dram_parameter("xt", [C, N], bf16, isOutput=False)
    # fused weight columns [K2|Q2 | K0|K1 | Q0|Q1]
    wkq_d = nc.declare_dram_parameter("wkq", [C, 2 * J], bf16,
                                      isOutput=False)
    wv_d = nc.declare_dram_parameter("wv", [C, J], bf16, isOutput=False)
    bqA_d = nc.declare_dram_parameter("bqA", [128, 1], f32, isOutput=False)
    bq2_d = nc.declare_dram_parameter("bq2", [64, 1], f32, isOutput=False)
    # padded proj weights: rows 0:128 = heads 0,1; 128:192 = head 2;
    # 192:256 = zero (annihilates ah2[1]'s junk bottom half)
    wp_d = nc.declare_dram_parameter("wp", [2 * 128, C], bf16,
                                     isOutput=False)
    out_d = nc.declare_dram_parameter("out", [N, C], bf16, isOutput=True)

    with tile.TileContext(nc) as tc:
        with (
            tc.tile_pool(name="persist", bufs=1) as pp,
            tc.tile_pool(name="osb", bufs=4) as posb,
            tc.tile_pool(name="etile", bufs=4) as pe,
            tc.tile_pool(name="bcsb", bufs=2) as pbc,
        ):
            warm_t = pp.tile([128, 512], bf16, tag="warm_t", name="warm_t")
            wkq = [pp.tile([128, 2 * J], bf16, tag=f"wkq{k}",
                           name=f"wkq{k}") for k in range(KC)]
            xt = [pp.tile([128, N], bf16, tag=f"xt{k}", name=f"xt{k}")
                  for k in range(KC)]
            wv = [pp.tile([128, J], bf16, tag=f"wv{k}", name=f"wv{k}")
                  for k in range(KC)]
            bqA = pp.tile([128, 1], f32, tag="bqA", name="bqA")
            bq2 = pp.tile([64, 1], f32, tag="bq2", name="bq2")
            wp = [pp.tile([128, C], bf16, tag=f"wp{t}", name=f"wp{t}")
                  for t in range(2)]
            # K^T per head, zero rows alternating so the packed Q tiles
            # need none: kh0 data 0:64, kh1 data 64:128, kh2 data 0:64
            kh = [pp.tile([128, N], bf16, tag=f"kh{h}", name=f"kh{h}")
                  for h in range(HPC)]
            # Q^T packed: qA = (Q0 top, Q1 bottom); qB = (Q2 top, junk)
            qA = pp.tile([128, N], bf16, tag="qA", name="qA")
            qB = pp.tile([128, N], bf16, tag="qB", name="qB")
            # V with a ones column per head: [128, 3*65]
            vx = [pp.tile([128, HPC * 65], bf16, tag=f"vx{m}",
                          name=f"vx{m}") for m in range(MC)]
            sums = [pp.tile([1, N], f32, tag=f"sums{h}", name=f"sums{h}")
                    for h in range(HPC)]
            raw = [pp.tile([64, N], bf16, tag=f"raw{h}", name=f"raw{h}")
                   for h in range(HPC)]
            ah2 = [pp.tile([128, N], bf16, tag=f"ah2{t}", name=f"ah2{t}")
                   for t in range(2)]

            # ---- constants via DVE memset (no DMA traffic) ----
            nc.vector.memset(warm_t[:], 1.0)
            nc.vector.memset(kh[0][64:128, :], 0.0)
            nc.vector.memset(kh[1][0:64, :], 0.0)
            nc.vector.memset(kh[2][64:128, :], 0.0)
            nc.vector.memset(qB[64:128, :], 0.0)
            nc.vector.memset(ah2[1][64:128, :], 0.0)
            for m in range(MC):
                on = vx[m].rearrange("p (h e) -> p h e", e=65)[:, :, 64:65]
                nc.vector.memset(on, 1.0)

            # ---- input DMA: x^T halves round-robin on two queues,
            # weights on the scalar queue ----
            dq = [nc.sync, nc.gpsimd]
            for half in range(2):
                csl = slice(1024 * half, 1024 * (half + 1))
                for k in range(KC):
                    dq[k % 2].dma_start(xt[k][:, csl],
                                        xt_d[128 * k:128 * (k + 1), csl])
            for t in range(2):
                nc.sync.dma_start(wp[t][:], wp_d[128 * t:128 * (t + 1), :])
            nc.scalar.dma_start(bqA[:], bqA_d[:, :])
            nc.scalar.dma_start(bq2[:], bq2_d[:, :])
            for k in range(KC):
                nc.scalar.dma_start(wkq[k][:], wkq_d[128 * k:128 * (k + 1), :])
            for k in range(KC):
                nc.scalar.dma_start(wv[k][:], wv_d[128 * k:128 * (k + 1), :])

            with tc.tile_pool(name="psum", bufs=1, space="PSUM") as psp:
                # PE p-state warmup during the DMA prologue
                for i in range(NWARM):
                    ps = psp.tile([128, 1024], f32, tag="s", bufs=2,
                                  name=f"warm{i}")
                    nc.tensor.matmul(ps[:, 0:512], warm_t[:, 0:128],
                                     warm_t[:])

                def g_group(blk, g):
                    # g0 -> [K2;Q2], g1 -> [K0;K1], g2 -> [Q0;Q1]
                    nsl = slice(1024 * blk, 1024 * (blk + 1))
                    ps = psp.tile([128, 1024], f32, tag="s", bufs=2,
                                  name="ps_g")
                    for i in range(2):
                        isl = slice(nsl.start + 512 * i,
                                    nsl.start + 512 * (i + 1))
                        for k in range(KC):
                            nc.tensor.matmul(
                                ps[:, 512 * i:512 * (i + 1)],
                                wkq[k][:, 128 * g:128 * (g + 1)],
                                xt[k][:, isl],
                                start=(k == 0), stop=(k == KC - 1))
                    if g == 0:
                        nc.vector.tensor_copy(kh[2][0:64, nsl], ps[0:64, :])
                        nc.vector.tensor_scalar(
                            qB[0:64, nsl], ps[64:128, :], 0.125,
                            bq2[:], mult, add)
                    elif g == 1:
                        nc.vector.tensor_copy(kh[0][0:64, nsl], ps[0:64, :])
                        nc.vector.tensor_copy(kh[1][64:128, nsl],
                                              ps[64:128, :])
                    else:
                        nc.vector.tensor_scalar(
                            qA[:, nsl], ps[:], 0.125, bqA[:], mult, add)

                def v_chunk(m):
                    msl = slice(128 * m, 128 * (m + 1))
                    ps = psp.tile([128, 512], f32, tag="v", bufs=2,
                                  name="ps_v")
                    for k in range(KC):
                        nc.tensor.matmul(ps[:, 0:J], xt[k][:, msl], wv[k][:],
                                         start=(k == 0), stop=(k == KC - 1))
                    vdst = vx[m].rearrange("p (h e) -> p h e",
                                           e=65)[:, :, 0:64]
                    nc.vector.tensor_copy(
                        vdst, ps[:, 0:J].rearrange("p (h e) -> p h e", e=64))

                def proj(m):
                    msl = slice(128 * m, 128 * (m + 1))
                    t = psp.tile([128, 1024], f32, tag="s", bufs=2,
                                 name="ps_pj")
                    for tt in range(2):
                        nc.tensor.matmul(t[:, 0:512], ah2[tt][:, msl],
                                         wp[tt][:, 0:512],
                                         start=(tt == 0), stop=(tt == 1))
                    for tt in range(2):
                        nc.tensor.matmul(t[:, 512:768], ah2[tt][:, msl],
                                         wp[tt][:, 512:768],
                                         start=(tt == 0), stop=(tt == 1))
                    o3 = posb.tile([128, C], bf16, tag="o3", name="o3")
                    if m % 2 == 0:
                        nc.vector.tensor_copy(o3[:], t[:, 0:768])
                    else:
                        nc.scalar.activation(o3[:], t[:, 0:768], Copy)
                    nc.sync.dma_start(out_d[msl, :], o3[:])

                pend = []

                def av_flush():
                    av, vsl, m, e = pend.pop(0)
                    for i in range(2):
                        nc.tensor.matmul(av[:, 512 * i:512 * (i + 1)],
                                         vx[m][:, vsl],
                                         e[:, 512 * i:512 * (i + 1)],
                                         start=(m == 0), stop=(m == MC - 1))

                def flush(h, cols, av, on_act, adst, r0):
                    # raw copy first (it alone gates the next pair's
                    # AV PSUM writes); the normalize chain runs off
                    # the critical path
                    if on_act:
                        nc.scalar.activation(raw[h][:, cols],
                                             av[0:64, :], Copy)
                    else:
                        nc.vector.tensor_copy(raw[h][:, cols],
                                              av[0:64, :])
                    nc.vector.tensor_copy(sums[h][:, cols], av[64:65, :])
                    for i in range(2):
                        hf = slice(cols.start + 512 * i,
                                   cols.start + 512 * (i + 1))
                        bcs = pbc.tile([64, 512], f32, tag="bcs",
                                       name="bcs")
                        nc.gpsimd.partition_broadcast(bcs[:],
                                                      sums[h][:, hf])
                        rec = pbc.tile([64, 512], f32, tag="rec",
                                       name="rec")
                        nc.vector.reciprocal_approx_fast(rec[:], bcs[:])
                        nc.vector.tensor_mul(adst[r0:r0 + 64, hf],
                                             raw[h][:, hf], rec[:])

                def attn_pair(h, nb, kt, qt, vsl, extra):
                    # extra: list of thunks to interleave, one after
                    # each of the first len(extra) chunks
                    q0 = 1024 * nb
                    av = psp.tile([65, 1024], f32, tag="av", bufs=1,
                                  name="ps_av")
                    for m in range(MC):
                        msl = slice(128 * m, 128 * (m + 1))
                        s = psp.tile([128, 1024], f32, tag="s", bufs=2,
                                     name="ps_s")
                        for i in range(2):
                            nc.tensor.matmul(s[:, 512 * i:512 * (i + 1)],
                                             kt[:, msl],
                                             qt[:, q0 + 512 * i:
                                                q0 + 512 * (i + 1)])
                        e = pe.tile([128, 1024], bf16, tag="e", name="e")
                        if m % 2 == 0:
                            nc.scalar.activation(e[:], s[:], Exp)
                        else:
                            nc.vector.tensor_scalar(e.bitcast(i16)[:], s[:],
                                                    SCH_A, SCH_B, mult, add)
                        pend.append((av, vsl, m, e))
                        if len(pend) > 1:
                            av_flush()
                        if m < len(extra):
                            extra[m]()
                    while pend:
                        av_flush()
                    return av

                v2 = slice(130, 195)
                v0 = slice(0, 65)
                v1 = slice(65, 130)
                c0 = slice(0, 1024)
                c1 = slice(1024, 2048)

                # front: QKV projections; V chunks 4..15 interleave
                # into the first attention pair
                for blk in range(2):
                    for g in range(3):
                        g_group(blk, g)
                for m in range(4):
                    v_chunk(m)

                ext1 = [(lambda mm: (lambda: v_chunk(mm)))(m)
                        for m in range(4, MC)]
                av = attn_pair(2, 0, kh[2], qB, v2, ext1)
                flush(2, c0, av, True, ah2[1], 0)
                av = attn_pair(0, 0, kh[0], qA, v0, [])
                flush(0, c0, av, False, ah2[0], 0)
                av = attn_pair(1, 0, kh[1], qA, v1, [])
                flush(1, c0, av, True, ah2[0], 64)
                # block-0 output projection rides inside pair (2,1)
                ext2 = [(lambda mm: (lambda: proj(mm)))(m)
                        for m in range(8)]
                av = attn_pair(2, 1, kh[2], qB, v2, ext2)
                flush(2, c1, av, False, ah2[1], 0)
                av = attn_pair(0, 1, kh[0], qA, v0, [])
                flush(0, c1, av, True, ah2[0], 0)
                av = attn_pair(1, 1, kh[1], qA, v1, [])
                flush(1, c1, av, False, ah2[0], 64)
                for m in range(8, MC):
                    proj(m)

    nc.compile()
    return nc


def kernel(x, w_qkv, b_qkv, w_proj, b_proj):
    import ml_dtypes

    from concourse.bass_utils import run_bass_kernel_spmd

    global LAST_RESULTS
    if "nc" not in _cache:
        _cache["nc"] = _build()
    nc = _cache["nc"]

    bf16 = ml_dtypes.bfloat16
    x = np.asarray(x, dtype=np.float32)
    w_qkv = np.asarray(w_qkv, dtype=np.float32)
    b_qkv = np.asarray(b_qkv, dtype=np.float32)
    w_proj = np.asarray(w_proj, dtype=np.float32)
    b_proj = np.asarray(b_proj, dtype=np.float32)

    in_maps = []
    for c in range(NCORES):
        b = c // 4
        h0 = HPC * (c % 4)
        q = [w_qkv[:, 64 * (h0 + h):64 * (h0 + h + 1)] for h in range(HPC)]
        k = [w_qkv[:, C + 64 * (h0 + h):C + 64 * (h0 + h + 1)]
             for h in range(HPC)]
        vs = slice(2 * C + 64 * h0, 2 * C + 64 * (h0 + HPC))
        wkq = np.concatenate([k[2], q[2], k[0], k[1], q[0], q[1]], axis=1)
        bq = b_qkv[64 * h0:64 * (h0 + HPC)] * 0.125
        wp_pad = np.zeros((2 * 128, C), dtype=np.float32)
        wp_pad[0:128] = w_proj[64 * h0:64 * (h0 + 2), :]
        wp_pad[128:192] = w_proj[64 * (h0 + 2):64 * (h0 + 3), :]
        in_maps.append({
            "xt": np.ascontiguousarray(x[b].T).astype(bf16),
            "wkq": np.ascontiguousarray(wkq).astype(bf16),
            "wv": np.ascontiguousarray(w_qkv[:, vs]).astype(bf16),
            "bqA": np.ascontiguousarray(bq[0:128].reshape(128, 1)).astype(
                np.float32),
            "bq2": np.ascontiguousarray(bq[128:192].reshape(64, 1)).astype(
                np.float32),
            "wp": wp_pad.astype(bf16),
        })

    res = run_bass_kernel_spmd(nc, in_maps, core_ids=list(range(NCORES)))
    LAST_RESULTS = res

    out = np.zeros((B, N, C), dtype=np.float32)
    for c in range(NCORES):
        out[c // 4] += np.asarray(res.results[c]["out"], dtype=np.float32)
    out += b_proj + b_qkv[2 * C:] @ w_proj
    return out
